# revision 6
# baseline (speedup 1.0000x reference)
"""Trainium2 Bass kernel for the autoregressive GRU decoder.

Problem: 512 sequential GRU steps over batch 4096, hidden 128; per step the
3-dim position output feeds back into the input.  Data-parallel over 8 cores
(512 batch rows per core), with the per-core batch split into 2 streams of
256 so the engines can pipeline across the sequential dependency chain.

Algebraic restructuring done on the host (validated vs fp64 golden):
  - pos_k = W_out h_k + b_out feeds the next step's input, so the input
    matmul folds into the hidden one: W_c = W_ih[:, :3] @ W_out + W_hh, with
    all z / bias contributions collapsed into one constant C per gate.
  - The gate pre-activations live persistently in PSUM.  They are
    initialized once with C (via an identity matmul) + W @ h, and every
    subsequent step only accumulates W_c @ delta where
    delta = h_new - h = (u - 1) * (h - n).  No per-step constant re-adds.
  - pos is computed with h-as-weights matmuls: out[128 batch, 3] =
    (h chunk)^T^T @ W_out^T, accumulated 128 steps per PSUM bank and
    evacuated in bulk -- output lands directly in [b, t, i] layout.

Layout per core: hidden state transposed hT [128(H), 512(B)] in SBUF.
PSUM banks: rz_A, nx_A, rz_B, nx_B (gate preacts, [r|z] and [xn|hn] halves)
plus 4 pos banks (one per 128-row batch chunk).
"""

import os
import numpy as np

B, H, LAT, IN = 4096, 128, 32, 3
NCORES = 8
BSH = B // NCORES          # 512 batch rows per core
NST = 2                    # streams per core
BST = BSH // NST           # 256 batch rows per stream
NCHUNK = BSH // 128        # 4 pos chunks of 128 batch rows
TWIN = 128                 # pos-accumulation window (steps per PSUM bank)

# "f32" | "f32r" | "bf16" for the per-step gate matmuls (delta path only;
# the init matmuls always run exact f32).
MM_DTYPE = os.environ.get("KERNEL_MM_DTYPE", "f32r")
# elementwise working dtype for t/s/n/ru/dd/delta tiles: "f32" | "bf16"
EW_DTYPE = os.environ.get("KERNEL_EW_DTYPE", "f32")

_CACHE = {}


def _host_prep(context, z, W_ih, W_hh, b_ih, b_hh, W_out, b_out):
    """Fold weights/constants; build per-core input maps."""
    f32 = np.float32
    sl = [slice(0, H), slice(H, 2 * H), slice(2 * H, 3 * H)]  # r, z, n rows

    Wp = (W_ih[:, :IN] @ W_out).astype(f32)           # pos feedback fold
    C0 = (W_ih[:, IN:] @ z.T + b_ih[:, None]).astype(f32)     # [384, B]
    C1 = (C0 + (W_ih[:, :IN] @ b_out)[:, None]).astype(f32)

    Wc_r = (Wp[sl[0]] + W_hh[sl[0]]).astype(f32)
    Wc_z = (Wp[sl[1]] + W_hh[sl[1]]).astype(f32)
    Wc_xn = Wp[sl[2]].astype(f32)
    W_hn = W_hh[sl[2]].astype(f32)

    def asc(a):
        return np.ascontiguousarray(a, dtype=f32)

    weights = {
        "w_r": asc(Wc_r.T), "w_z": asc(Wc_z.T),
        "w_xn": asc(Wc_xn.T), "w_hn": asc(W_hn.T),
        "w0_r": asc(W_hh[sl[0]].T), "w0_z": asc(W_hh[sl[1]].T),
        "w_out_t": asc(W_out.T),                       # [128, 3]
        "ident": np.eye(H, dtype=f32),
        "pos_bias": asc(np.tile(b_out, (128, TWIN))),  # [128, TWIN*IN]
    }

    in_maps = []
    for c in range(NCORES):
        bs = slice(c * BSH, (c + 1) * BSH)
        m = dict(weights)
        m["h0t"] = asc(context[bs].T)                  # [H, BSH]
        for X in range(NST):
            cs = slice(c * BSH + X * BST, c * BSH + (X + 1) * BST)
            bhh = b_hh[:, None]
            m[f"c0_rz_{X}"] = asc(np.concatenate(
                [C0[sl[0], cs] + bhh[sl[0]], C0[sl[1], cs] + bhh[sl[1]]], axis=1))
            m[f"c1_rz_{X}"] = asc(np.concatenate(
                [C1[sl[0], cs] + bhh[sl[0]], C1[sl[1], cs] + bhh[sl[1]]], axis=1))
            hn_const = np.broadcast_to(b_hh[sl[2]][:, None], (H, BST))
            m[f"c0_nx_{X}"] = asc(np.concatenate([C0[sl[2], cs], hn_const], axis=1))
            m[f"c1_nx_{X}"] = asc(np.concatenate([C1[sl[2], cs], hn_const], axis=1))
        in_maps.append(m)
    return in_maps


def _build(steps):
    import concourse.bacc as bacc
    import concourse.mybir as mybir
    from concourse.tile import TileContext

    f32 = mybir.dt.float32
    bf16 = mybir.dt.bfloat16
    Act = mybir.ActivationFunctionType
    Op = mybir.AluOpType

    ew_dt = bf16 if EW_DTYPE == "bf16" else f32

    def mmcast(ap):
        if MM_DTYPE == "f32r":
            return ap.bitcast(mybir.dt.float32r)
        return ap

    nc = bacc.Bacc("TRN2", target_bir_lowering=False, debug=False)

    # ---- DRAM parameters ----
    names_2d = ["h0t"] + [f"c{i}_{g}_{X}" for i in (0, 1)
                          for g in ("rz", "nx") for X in range(NST)]
    params = {}
    for n in names_2d:
        params[n] = nc.declare_dram_parameter(n, [H, BSH], f32, isOutput=False)
    for n in ("w_r", "w_z", "w_xn", "w_hn", "w0_r", "w0_z", "ident"):
        params[n] = nc.declare_dram_parameter(n, [H, H], f32, isOutput=False)
    params["w_out_t"] = nc.declare_dram_parameter("w_out_t", [H, IN], f32,
                                                  isOutput=False)
    params["pos_bias"] = nc.declare_dram_parameter("pos_bias", [128, TWIN * IN],
                                                   f32, isOutput=False)
    p_out = nc.declare_dram_parameter("out", [BSH, steps, IN], f32,
                                      isOutput=True)

    mm_wt_dt = bf16 if MM_DTYPE == "bf16" else f32

    with TileContext(nc) as tc, \
            tc.tile_pool(name="const", bufs=1) as cpool, \
            tc.tile_pool(name="state", bufs=1) as spool, \
            tc.tile_pool(name="work", bufs=3) as wpool, \
            tc.tile_pool(name="stage", bufs=2) as stpool, \
            tc.tile_pool(name="psum", bufs=1, space="PSUM") as ppool:

        sb = {}
        for n in names_2d + ["w_r", "w_z", "w_xn", "w_hn", "w0_r", "w0_z",
                             "ident", "w_out_t", "pos_bias"]:
            t = cpool.tile(list(params[n].shape), f32, name=f"sb_{n}")
            nc.sync.dma_start(out=t[:], in_=params[n][:])
            sb[n] = t

        # per-step matmul weights, possibly downcast to bf16 once
        stepw = {}
        for n in ("w_r", "w_z", "w_xn", "w_hn"):
            if MM_DTYPE == "bf16":
                t = cpool.tile([H, H], bf16, name=f"bw_{n}")
                nc.vector.tensor_copy(t[:], sb[n][:])
                stepw[n] = t
            else:
                stepw[n] = sb[n]

        h = spool.tile([H, BSH], f32, name="h")
        nc.sync.dma_start(out=h[:], in_=params["h0t"][:])

        rz = [ppool.tile([128, 2 * BST], f32, name=f"ps_rz{X}")
              for X in range(NST)]
        nx = [ppool.tile([128, 2 * BST], f32, name=f"ps_nx{X}")
              for X in range(NST)]
        posb = [ppool.tile([128, TWIN * IN], f32, name=f"ps_pos{c}")
                for c in range(NCHUNK)]

        delta_prev = [None, None]

        for k in range(steps):
            mode = 0 if k == 0 else (1 if k == 1 else 2)

            # ---- PE: gate matmuls ----
            if mode < 2:
                ci = f"c{mode}"
                for X in range(NST):
                    nc.tensor.matmul(rz[X][:], sb["ident"][:],
                                     sb[f"{ci}_rz_{X}"][:],
                                     start=True, stop=False)
                    nc.tensor.matmul(nx[X][:], sb["ident"][:],
                                     sb[f"{ci}_nx_{X}"][:],
                                     start=True, stop=(mode == 0))
                wr = sb["w0_r"] if mode == 0 else sb["w_r"]
                wz = sb["w0_z"] if mode == 0 else sb["w_z"]
                gate_mms = [(wr, rz, 0, False), (wz, rz, 1, True),
                            (sb["w_hn"], nx, 1, True)]
                if mode == 1:
                    gate_mms.insert(2, (sb["w_xn"], nx, 0, False))
                for w, bank, half, stop in gate_mms:
                    for X in range(NST):
                        nc.tensor.matmul(
                            bank[X][:, half * BST:(half + 1) * BST],
                            w[:], h[:, X * BST:(X + 1) * BST],
                            start=False, stop=stop)
            else:
                for w, bank, half, stop in [
                        (stepw["w_r"], rz, 0, False),
                        (stepw["w_z"], rz, 1, True),
                        (stepw["w_xn"], nx, 0, False),
                        (stepw["w_hn"], nx, 1, True)]:
                    for X in range(NST):
                        nc.tensor.matmul(
                            bank[X][:, half * BST:(half + 1) * BST],
                            mmcast(w[:]), mmcast(delta_prev[X][:]),
                            start=False, stop=stop)

            # ---- gates / state update, engine-interleaved across streams ----
            ru = [wpool.tile([128, 2 * BST], ew_dt, name=f"ru{X}", bufs=2)
                  for X in range(NST)]
            for X in range(NST):
                nc.scalar.activation(ru[X][:], rz[X][:], Act.Sigmoid)

            ts = []
            for X in range(NST):
                t = wpool.tile([128, BST], ew_dt, name=f"t{X}", bufs=2)
                nc.vector.tensor_tensor(t[:], nx[X][:, BST:2 * BST],
                                        ru[X][:, 0:BST], Op.mult)
                s_ = wpool.tile([128, BST], ew_dt, name=f"s{X}", bufs=2)
                nc.vector.tensor_tensor(s_[:], t[:], nx[X][:, 0:BST], Op.add)
                ts.append(s_)

            nn = []
            for X in range(NST):
                n_ = wpool.tile([128, BST], ew_dt, name=f"n{X}", bufs=2)
                nc.scalar.activation(n_[:], ts[X][:], Act.Tanh)
                nn.append(n_)

            dds = []
            for X in range(NST):
                dd = wpool.tile([128, BST], ew_dt, name=f"dd{X}", bufs=2)
                nc.gpsimd.tensor_tensor(dd[:], h[:, X * BST:(X + 1) * BST],
                                        nn[X][:], Op.subtract)
                dds.append(dd)

            mm_delta_dt = bf16 if MM_DTYPE == "bf16" else f32
            deltas = []
            for X in range(NST):
                d = wpool.tile([128, BST], mm_delta_dt, name=f"d{X}", bufs=2)
                nc.vector.scalar_tensor_tensor(d[:], ru[X][:, BST:2 * BST],
                                               1.0, dds[X][:],
                                               Op.subtract, Op.mult)
                deltas.append(d)
            delta_prev = deltas

            for X in range(NST):
                nc.gpsimd.tensor_tensor(h[:, X * BST:(X + 1) * BST],
                                        h[:, X * BST:(X + 1) * BST],
                                        deltas[X][:], Op.add)

            # ---- PE: pos output matmuls (h chunk as stationary operand) ----
            j = k % TWIN
            for c in range(NCHUNK):
                nc.tensor.matmul(posb[c][:, IN * j:IN * (j + 1)],
                                 h[:, c * 128:(c + 1) * 128],
                                 sb["w_out_t"][:], start=True, stop=True)

            # ---- window flush: bias-add + DMA out ----
            if j == TWIN - 1 or k == steps - 1:
                t0 = k - j
                w = (j + 1) * IN
                for c in range(NCHUNK):
                    stg = stpool.tile([128, TWIN * IN], f32, name=f"stg{c}")
                    nc.vector.tensor_tensor(stg[:, :w], posb[c][:, :w],
                                            sb["pos_bias"][:, :w], Op.add)
                    nc.sync.dma_start(
                        out=p_out[c * 128:(c + 1) * 128, t0:t0 + j + 1, :],
                        in_=stg[:, :w].rearrange("p (t i) -> p t i", i=IN))

    nc.finalize()
    return nc


def _get_nc(steps):
    key = (steps, MM_DTYPE, EW_DTYPE)
    if key not in _CACHE:
        _CACHE[key] = _build(steps)
    return _CACHE[key]


def kernel(context, z, steps, W_ih, W_hh, b_ih, b_hh, W_out, b_out):
    from concourse.bass_utils import run_bass_kernel_spmd

    context = np.asarray(context, dtype=np.float32)
    z = np.asarray(z, dtype=np.float32)
    W_ih = np.asarray(W_ih, dtype=np.float32)
    W_hh = np.asarray(W_hh, dtype=np.float32)
    b_ih = np.asarray(b_ih, dtype=np.float32)
    b_hh = np.asarray(b_hh, dtype=np.float32)
    W_out = np.asarray(W_out, dtype=np.float32)
    b_out = np.asarray(b_out, dtype=np.float32)
    steps = int(steps)
    assert context.shape == (B, H) and z.shape == (B, LAT)

    nc = _get_nc(steps)
    in_maps = _host_prep(context, z, W_ih, W_hh, b_ih, b_hh, W_out, b_out)
    res = run_bass_kernel_spmd(nc, in_maps, core_ids=list(range(NCORES)))
    out = np.concatenate([res.results[c]["out"] for c in range(NCORES)], axis=0)
    return out


# revision 10
# speedup vs baseline: 1.9073x; 1.9073x over previous
"""Trainium2 Bass kernel for the autoregressive GRU decoder.

Problem: 512 sequential GRU steps over batch 4096, hidden 128; per step the
3-dim position output feeds back into the input.  Data-parallel over 8 cores
(512 batch rows per core), with the per-core batch split into 2 streams of
256 so the engines can pipeline across the sequential dependency chain.

Algebraic restructuring done on the host (validated vs fp64 golden):
  - pos_k = W_out h_k + b_out feeds the next step's input, so the input
    matmul folds into the hidden one: W_c = W_ih[:, :3] @ W_out + W_hh, with
    all z / bias contributions collapsed into one constant C per gate.
  - The gate pre-activations live persistently in PSUM.  They are
    initialized once with C (via an identity matmul) + W @ h, and every
    subsequent step only accumulates W_c @ delta where
    delta = h_new - h = (u - 1) * (h - n).  No per-step constant re-adds.
  - pos is computed with h-as-weights matmuls: out[128 batch, 3] =
    (h chunk)^T^T @ W_out^T, accumulated 128 steps per PSUM bank and
    evacuated in bulk -- output lands directly in [b, t, i] layout.

Layout per core: hidden state transposed hT [128(H), 512(B)] in SBUF.
PSUM banks: rz_A, nx_A, rz_B, nx_B (gate preacts, [r|z] and [xn|hn] halves)
plus 4 pos banks (one per 128-row batch chunk).
"""

import os
import numpy as np

B, H, LAT, IN = 4096, 128, 32, 3
NCORES = 8
BSH = B // NCORES          # 512 batch rows per core
NST = 2                    # streams per core
BST = BSH // NST           # 256 batch rows per stream
NCHUNK = BSH // 128        # 4 pos chunks of 128 batch rows
TWIN = 128                 # pos-accumulation window (steps per PSUM bank)

# "f32" | "f32r" | "bf16" for the per-step gate matmuls (delta path only;
# the init matmuls always run exact f32).
MM_DTYPE = os.environ.get("KERNEL_MM_DTYPE", "f32r")
# elementwise working dtype for t/s/n/ru/dd/delta tiles: "f32" | "bf16"
EW_DTYPE = os.environ.get("KERNEL_EW_DTYPE", "f32")

_CACHE = {}


def _host_prep(context, z, W_ih, W_hh, b_ih, b_hh, W_out, b_out):
    """Fold weights/constants; build per-core input maps."""
    f32 = np.float32
    sl = [slice(0, H), slice(H, 2 * H), slice(2 * H, 3 * H)]  # r, z, n rows

    Wp = (W_ih[:, :IN] @ W_out).astype(f32)           # pos feedback fold
    C0 = (W_ih[:, IN:] @ z.T + b_ih[:, None]).astype(f32)     # [384, B]
    C1 = (C0 + (W_ih[:, :IN] @ b_out)[:, None]).astype(f32)

    Wc_r = (Wp[sl[0]] + W_hh[sl[0]]).astype(f32)
    Wc_z = (Wp[sl[1]] + W_hh[sl[1]]).astype(f32)
    Wc_xn = Wp[sl[2]].astype(f32)
    W_hn = W_hh[sl[2]].astype(f32)

    def asc(a):
        return np.ascontiguousarray(a, dtype=f32)

    weights = {
        "w_r": asc(Wc_r.T), "w_z": asc(Wc_z.T),
        "w_xn": asc(Wc_xn.T), "w_hn": asc(W_hn.T),
        "w0_r": asc(W_hh[sl[0]].T), "w0_z": asc(W_hh[sl[1]].T),
        "w_out_t": asc(W_out.T),                       # [128, 3]
        "ident": np.eye(H, dtype=f32),
        "pos_bias": asc(np.tile(b_out, (128, TWIN))),  # [128, TWIN*IN]
    }

    in_maps = []
    for c in range(NCORES):
        bs = slice(c * BSH, (c + 1) * BSH)
        m = dict(weights)
        m["h0t"] = asc(context[bs].T)                  # [H, BSH]
        for X in range(NST):
            cs = slice(c * BSH + X * BST, c * BSH + (X + 1) * BST)
            bhh = b_hh[:, None]
            m[f"c0_rz_{X}"] = asc(np.concatenate(
                [C0[sl[0], cs] + bhh[sl[0]], C0[sl[1], cs] + bhh[sl[1]]], axis=1))
            m[f"c1_rz_{X}"] = asc(np.concatenate(
                [C1[sl[0], cs] + bhh[sl[0]], C1[sl[1], cs] + bhh[sl[1]]], axis=1))
            hn_const = np.broadcast_to(b_hh[sl[2]][:, None], (H, BST))
            m[f"c0_nx_{X}"] = asc(np.concatenate([C0[sl[2], cs], hn_const], axis=1))
            m[f"c1_nx_{X}"] = asc(np.concatenate([C1[sl[2], cs], hn_const], axis=1))
        in_maps.append(m)
    return in_maps


def _build(steps):
    import concourse.bacc as bacc
    import concourse.mybir as mybir
    from concourse.tile import TileContext

    f32 = mybir.dt.float32
    bf16 = mybir.dt.bfloat16
    Act = mybir.ActivationFunctionType
    Op = mybir.AluOpType

    ew_dt = bf16 if EW_DTYPE == "bf16" else f32
    # dtype for the per-step gate matmul operands (weights + delta).
    mm_dt = {"f32": f32, "f32r": mybir.dt.float32r, "bf16": bf16}[MM_DTYPE]

    nc = bacc.Bacc("TRN2", target_bir_lowering=False, debug=False)

    # ---- DRAM parameters ----
    names_2d = ["h0t"] + [f"c{i}_{g}_{X}" for i in (0, 1)
                          for g in ("rz", "nx") for X in range(NST)]
    params = {}
    for n in names_2d:
        params[n] = nc.declare_dram_parameter(n, [H, BSH], f32, isOutput=False)
    for n in ("w_r", "w_z", "w_xn", "w_hn", "w0_r", "w0_z", "ident"):
        params[n] = nc.declare_dram_parameter(n, [H, H], f32, isOutput=False)
    params["w_out_t"] = nc.declare_dram_parameter("w_out_t", [H, IN], f32,
                                                  isOutput=False)
    params["pos_bias"] = nc.declare_dram_parameter("pos_bias", [128, TWIN * IN],
                                                   f32, isOutput=False)
    p_out = nc.declare_dram_parameter("out", [BSH, steps, IN], f32,
                                      isOutput=True)

    mm_wt_dt = bf16 if MM_DTYPE == "bf16" else f32

    with TileContext(nc) as tc, \
            tc.tile_pool(name="const", bufs=1) as cpool, \
            tc.tile_pool(name="state", bufs=1) as spool, \
            tc.tile_pool(name="work", bufs=3) as wpool, \
            tc.tile_pool(name="stage", bufs=2) as stpool, \
            tc.tile_pool(name="psum", bufs=1, space="PSUM") as ppool:

        sb = {}
        for n in names_2d + ["w_r", "w_z", "w_xn", "w_hn", "w0_r", "w0_z",
                             "ident", "w_out_t", "pos_bias"]:
            t = cpool.tile(list(params[n].shape), f32, name=f"sb_{n}")
            nc.sync.dma_start(out=t[:], in_=params[n][:])
            sb[n] = t

        # per-step matmul weights, converted to the mm dtype once
        stepw = {}
        for n in ("w_r", "w_z", "w_xn", "w_hn"):
            if MM_DTYPE != "f32":
                t = cpool.tile([H, H], mm_dt, name=f"cw_{n}")
                nc.vector.tensor_copy(t[:], sb[n][:])
                stepw[n] = t
            else:
                stepw[n] = sb[n]

        h = spool.tile([H, BSH], f32, name="h")
        nc.sync.dma_start(out=h[:], in_=params["h0t"][:])

        rz = [ppool.tile([128, 2 * BST], f32, name=f"ps_rz{X}")
              for X in range(NST)]
        nx = [ppool.tile([128, 2 * BST], f32, name=f"ps_nx{X}")
              for X in range(NST)]
        posb = [ppool.tile([128, TWIN * IN], f32, name=f"ps_pos{c}")
                for c in range(NCHUNK)]

        delta_prev = [None, None]

        for k in range(steps):
            mode = 0 if k == 0 else (1 if k == 1 else 2)

            # ---- PE: gate matmuls ----
            if mode < 2:
                ci = f"c{mode}"
                for X in range(NST):
                    nc.tensor.matmul(rz[X][:], sb["ident"][:],
                                     sb[f"{ci}_rz_{X}"][:],
                                     start=True, stop=False)
                    nc.tensor.matmul(nx[X][:], sb["ident"][:],
                                     sb[f"{ci}_nx_{X}"][:],
                                     start=True, stop=(mode == 0))
                wr = sb["w0_r"] if mode == 0 else sb["w_r"]
                wz = sb["w0_z"] if mode == 0 else sb["w_z"]
                gate_mms = [(wr, rz, 0, False), (wz, rz, 1, True),
                            (sb["w_hn"], nx, 1, True)]
                if mode == 1:
                    gate_mms.insert(2, (sb["w_xn"], nx, 0, False))
                for w, bank, half, stop in gate_mms:
                    for X in range(NST):
                        nc.tensor.matmul(
                            bank[X][:, half * BST:(half + 1) * BST],
                            w[:], h[:, X * BST:(X + 1) * BST],
                            start=False, stop=stop)
            else:
                for w, bank, half, stop in [
                        (stepw["w_r"], rz, 0, False),
                        (stepw["w_z"], rz, 1, True),
                        (stepw["w_xn"], nx, 0, False),
                        (stepw["w_hn"], nx, 1, True)]:
                    for X in range(NST):
                        nc.tensor.matmul(
                            bank[X][:, half * BST:(half + 1) * BST],
                            w[:], delta_prev[X][:],
                            start=False, stop=stop)

            # ---- gates / state update, engine-interleaved across streams ----
            ru = [wpool.tile([128, 2 * BST], ew_dt, name=f"ru{X}", bufs=2)
                  for X in range(NST)]
            for X in range(NST):
                nc.scalar.activation(ru[X][:], rz[X][:], Act.Sigmoid)

            ts = []
            for X in range(NST):
                t = wpool.tile([128, BST], ew_dt, name=f"t{X}", bufs=2)
                nc.vector.tensor_tensor(t[:], nx[X][:, BST:2 * BST],
                                        ru[X][:, 0:BST], Op.mult)
                s_ = wpool.tile([128, BST], ew_dt, name=f"s{X}", bufs=2)
                nc.vector.tensor_tensor(s_[:], t[:], nx[X][:, 0:BST], Op.add)
                ts.append(s_)

            nn = []
            for X in range(NST):
                n_ = wpool.tile([128, BST], ew_dt, name=f"n{X}", bufs=2)
                nc.scalar.activation(n_[:], ts[X][:], Act.Tanh)
                nn.append(n_)

            dds = []
            for X in range(NST):
                dd = wpool.tile([128, BST], ew_dt, name=f"dd{X}", bufs=2)
                nc.gpsimd.tensor_tensor(dd[:], h[:, X * BST:(X + 1) * BST],
                                        nn[X][:], Op.subtract)
                dds.append(dd)

            deltas = []
            for X in range(NST):
                d = wpool.tile([128, BST], mm_dt, name=f"d{X}", bufs=2)
                nc.vector.scalar_tensor_tensor(d[:], ru[X][:, BST:2 * BST],
                                               1.0, dds[X][:],
                                               Op.subtract, Op.mult)
                deltas.append(d)
            delta_prev = deltas

            for X in range(NST):
                nc.gpsimd.tensor_tensor(h[:, X * BST:(X + 1) * BST],
                                        h[:, X * BST:(X + 1) * BST],
                                        deltas[X][:], Op.add)

            # ---- PE: pos output matmuls (h chunk as stationary operand) ----
            j = k % TWIN
            for c in range(NCHUNK):
                nc.tensor.matmul(posb[c][:, IN * j:IN * (j + 1)],
                                 h[:, c * 128:(c + 1) * 128],
                                 sb["w_out_t"][:], start=True, stop=True)

            # ---- window flush: bias-add + DMA out ----
            if j == TWIN - 1 or k == steps - 1:
                t0 = k - j
                w = (j + 1) * IN
                for c in range(NCHUNK):
                    stg = stpool.tile([128, TWIN * IN], f32, name=f"stg{c}")
                    nc.vector.tensor_tensor(stg[:, :w], posb[c][:, :w],
                                            sb["pos_bias"][:, :w], Op.add)
                    nc.sync.dma_start(
                        out=p_out[c * 128:(c + 1) * 128, t0:t0 + j + 1, :],
                        in_=stg[:, :w].rearrange("p (t i) -> p t i", i=IN))

    nc.finalize()
    return nc


def _get_nc(steps):
    key = (steps, MM_DTYPE, EW_DTYPE)
    if key not in _CACHE:
        _CACHE[key] = _build(steps)
    return _CACHE[key]


def kernel(context, z, steps, W_ih, W_hh, b_ih, b_hh, W_out, b_out):
    from concourse.bass_utils import run_bass_kernel_spmd

    context = np.asarray(context, dtype=np.float32)
    z = np.asarray(z, dtype=np.float32)
    W_ih = np.asarray(W_ih, dtype=np.float32)
    W_hh = np.asarray(W_hh, dtype=np.float32)
    b_ih = np.asarray(b_ih, dtype=np.float32)
    b_hh = np.asarray(b_hh, dtype=np.float32)
    W_out = np.asarray(W_out, dtype=np.float32)
    b_out = np.asarray(b_out, dtype=np.float32)
    steps = int(steps)
    assert context.shape == (B, H) and z.shape == (B, LAT)

    nc = _get_nc(steps)
    in_maps = _host_prep(context, z, W_ih, W_hh, b_ih, b_hh, W_out, b_out)
    res = run_bass_kernel_spmd(nc, in_maps, core_ids=list(range(NCORES)))
    out = np.concatenate([res.results[c]["out"] for c in range(NCORES)], axis=0)
    return out
